# revision 7
# baseline (speedup 1.0000x reference)
import sys
sys.path.insert(0, "/opt/trn_rl_repo")
import math
import os
import numpy as np
import ml_dtypes

import concourse.bacc as bacc
import concourse.bass as bass
import concourse.mybir as mybir
import concourse.tile as tile
from concourse.bass_utils import run_bass_kernel_spmd
from concourse.masks import make_identity

bf16 = ml_dtypes.bfloat16
F32 = mybir.dt.float32
BF16 = mybir.dt.bfloat16
I16 = mybir.dt.int16

N = 50000
E = 800000
IN = 512
H1, D1 = 4, 64
HD1 = 256
H2, D2 = 1, 64
NCORES = 8
NSH = N // NCORES          # 6250 nodes per core
P = 128
NBLK = math.ceil(NSH / P)  # 49
LO = 32768                 # int16 gather index limit split
GCH = int(os.environ.get("K_GCH", "4"))
SP = bool(int(os.environ.get("K_SP", "1")))
RW1 = 384                  # T1 row width in bf16: 256 feat | 4 el f32 | pad (768B rows)
RW2 = 128                  # T2 row width in bf16: 64 feat | el f32 | pad (256B rows)


def _wrap16(idx):
    """[n] ints -> [128, n//16] int16 gather-index layout (16-partition wrap, x8 replicated)."""
    n = len(idx)
    assert n % 16 == 0
    a = np.asarray(idx, dtype=np.int16).reshape(n // 16, 16).T
    return np.tile(a, (8, 1))


def _prep_edges(src, dst):
    """Host-side edge sharding/ordering. Returns per-device index arrays + global schedule."""
    src = np.asarray(src).astype(np.int64)
    dst = np.asarray(dst).astype(np.int64)

    dev_lists = []  # [d][b] -> (lo_src, lo_dstoff, hi_src, hi_dstoff)
    for d in range(NCORES):
        m = (dst >= NSH * d) & (dst < NSH * (d + 1))
        s_d = src[m]
        t_d = dst[m] - NSH * d
        o = np.argsort(t_d, kind="stable")
        s_d, t_d = s_d[o], t_d[o]
        blk = t_d // P
        islo = s_d < LO
        blocks = []
        for b in range(NBLK):
            mb = blk == b
            sl, tl = s_d[mb & islo], t_d[mb & islo]
            sh, th = s_d[mb & ~islo], t_d[mb & ~islo]
            blocks.append((sl, tl - P * b, sh, th - P * b))
        dev_lists.append(blocks)

    nA = np.zeros(NBLK, dtype=np.int64)
    nB = np.zeros(NBLK, dtype=np.int64)
    for b in range(NBLK):
        for d in range(NCORES):
            sl, _, sh, _ = dev_lists[d][b]
            nA[b] = max(nA[b], (len(sl) + P - 1) // P)
            nB[b] = max(nB[b], (len(sh) + P - 1) // P)
        if nA[b] + nB[b] == 0:
            nA[b] = 1
    T = nA + nB
    NT = int(T.sum())

    idx_lo, idx_hi, doff = [], [], []
    for d in range(NCORES):
        lo_cols, hi_cols, do_cols = [], [], []
        for b in range(NBLK):
            sl, ol, sh, oh = dev_lists[d][b]
            npadA = nA[b] * P - len(sl)
            npadB = nB[b] * P - len(sh)
            lo_i = np.concatenate([sl, np.zeros(npadA, np.int64)])
            lo_o = np.concatenate([ol, np.full(npadA, -1.0)])
            hi_i = np.concatenate([sh - LO, np.zeros(npadB, np.int64)])
            hi_o = np.concatenate([oh, np.full(npadB, -1.0)])
            if nA[b]:
                lo_cols.append(_wrap16(lo_i))
            if nB[b]:
                hi_cols.append(_wrap16(hi_i))
            do = np.concatenate([lo_o, hi_o]).astype(np.float32)
            do_cols.append(do.reshape(T[b], P).T)
        idx_lo.append(np.hstack(lo_cols).astype(np.int16) if lo_cols else np.zeros((128, 0), np.int16))
        idx_hi.append(np.hstack(hi_cols).astype(np.int16) if hi_cols else np.zeros((128, 0), np.int16))
        doff.append(np.hstack(do_cols).astype(bf16))
    return nA, nB, NT, idx_lo, idx_hi, doff


def _build(nA, nB, NT, CL, CH, has_b1, has_b2):
    STAGE = int(os.environ.get("K_STAGE", "6"))
    NQ = int(os.environ.get("K_QUEUES", "4"))
    STC = bool(int(os.environ.get("K_STC", "1")))
    EBUFS = int(os.environ.get("K_EBUFS", "2"))
    nc = bacc.Bacc("TRN2", target_bir_lowering=False, debug=False, num_devices=NCORES,
                   num_swdge_queues=NQ)
    qctr = [0]
    def nextq():
        q = qctr[0] % NQ
        qctr[0] += 1
        return q

    xT = nc.dram_tensor("xT", [IN, NSH], F32, kind="ExternalInput")
    w1 = nc.dram_tensor("w1", [IN, HD1], F32, kind="ExternalInput")
    w1t = nc.dram_tensor("w1t", [HD1, IN], F32, kind="ExternalInput")
    alar1 = nc.dram_tensor("alar1", [HD1, 8], F32, kind="ExternalInput")
    w2 = nc.dram_tensor("w2", [HD1, D2], F32, kind="ExternalInput")
    w2t = nc.dram_tensor("w2t", [D2, HD1], F32, kind="ExternalInput")
    alar2 = nc.dram_tensor("alar2", [D2, 2], F32, kind="ExternalInput")
    ilo = nc.dram_tensor("ilo", [128, max(CL, 1)], I16, kind="ExternalInput")
    ihi = nc.dram_tensor("ihi", [128, max(CH, 1)], I16, kind="ExternalInput")
    idoff = nc.dram_tensor("idoff", [128, NT], BF16, kind="ExternalInput")
    if has_b1:
        b1r = nc.dram_tensor("b1r", [128, HD1], F32, kind="ExternalInput")
    if has_b2:
        b2r = nc.dram_tensor("b2r", [128, D2], F32, kind="ExternalInput")
    out_t = nc.dram_tensor("out", [NSH, D2], F32, kind="ExternalOutput")

    iota_np = np.tile(np.arange(128, dtype=bf16)[None, :], (128, 1))
    iota_d = nc.inline_tensor(iota_np, name="iota_c")

    ps_last = NSH - P * (NBLK - 1)  # rows in last block (106)

    with tile.TileContext(nc) as tc:
        with (
            tc.tile_pool(name="const", bufs=1) as cpool,
            tc.tile_pool(name="dram", bufs=1, space="DRAM") as dram,
        ):
            iota_t = cpool.tile([128, 128], BF16)
            nc.sync.dma_start(out=iota_t[:], in_=iota_d[:, :])
            ident = cpool.tile([128, 128], BF16)
            make_identity(nc, ident[:])

            ilo_t = cpool.tile([128, max(CL, 1)], I16)
            ihi_t = cpool.tile([128, max(CH, 1)], I16)
            doff_t = cpool.tile([128, NT], BF16)
            nc.sync.dma_start(out=ilo_t[:], in_=ilo[:, :])
            nc.sync.dma_start(out=ihi_t[:], in_=ihi[:, :])
            nc.sync.dma_start(out=doff_t[:], in_=idoff[:, :])
            if has_b1:
                b1_t = cpool.tile([128, HD1], F32)
                nc.sync.dma_start(out=b1_t[:], in_=b1r[:, :])
            if has_b2:
                b2_t = cpool.tile([128, D2], F32)
                nc.sync.dma_start(out=b2_t[:], in_=b2r[:, :])

            # persistent tiles: hT [feat(2x128 chunks), node] layout [b][k][128]
            hT = cpool.tile([128, NBLK * 256], BF16, tag="hT", name="hT")
            er1_sb = cpool.tile([128, NBLK * 4], BF16, tag="er1", name="er1")
            er2_sb = cpool.tile([128, NBLK], BF16, tag="er2", name="er2")
            # zero so garbage rows (beyond pb of last block) can't inject NaN/Inf
            # into the one-hot er expansion matmuls
            nc.vector.memset(er1_sb[:], 0)
            nc.vector.memset(er2_sb[:], 0)

            T1_local = dram.tile([NSH, RW1], BF16)
            T1_full = dram.tile([N, RW1], BF16, addr_space="Shared")
            T2_local = dram.tile([NSH, RW2], BF16)
            T2_full = dram.tile([N, RW2], BF16, addr_space="Shared")

            # ---------------- phase 0+1: dense L1 (feat1/el1 -> T1_local, er1 -> SBUF) ----
            with (
                tc.tile_pool(name="dsb", bufs=1) as dsb,
                tc.tile_pool(name="dps", bufs=2, space="PSUM") as dps,
                tc.tile_pool(name="combop", bufs=3) as combop,
            ):
                w1t_t = []
                for k in range(2):
                    w1t_k = dsb.tile([128, IN], BF16, tag=f"w1t{k}", name=f"w1t{k}")
                    w1t_t.append(w1t_k)
                alar1_t = []
                for k in range(2):
                    alar1_k = dsb.tile([128, 8], BF16, tag=f"alar1{k}", name=f"alar1{k}")
                    alar1_t.append(alar1_k)
                for k in range(2):
                    nc.gpsimd.dma_start(out=w1t_t[k][:], in_=w1t[128 * k:128 * (k + 1), :])
                    nc.gpsimd.dma_start(out=alar1_t[k][:], in_=alar1[128 * k:128 * (k + 1), :])
                rhsW1 = []
                for k in range(4):
                    rhsW1_k = dsb.tile([128, 264], BF16, tag=f"rhsW1{k}", name=f"rhsW1{k}")
                    rhsW1.append(rhsW1_k)
                for k in range(4):
                    nc.gpsimd.dma_start(out=rhsW1[k][:, 0:256], in_=w1[128 * k:128 * (k + 1), :])
                    psw = dps.tile([128, 8], F32, tag="psw")
                    for k2 in range(2):
                        nc.tensor.matmul(
                            out=psw[:], lhsT=w1t_t[k2][:, 128 * k:128 * (k + 1)],
                            rhs=alar1_t[k2][:], start=(k2 == 0), stop=(k2 == 1))
                    nc.vector.tensor_copy(rhsW1[k][:, 256:264], psw[:])

                xT_t = []
                for k in range(4):
                    xT_k = dsb.tile([128, NSH], BF16, tag=f"xT{k}", name=f"xT{k}")
                    xT_t.append(xT_k)
                for k in range(4):
                    nc.gpsimd.dma_start(out=xT_t[k][:], in_=xT[128 * k:128 * (k + 1), :])

                for nb in range(NBLK):
                    pb = P if nb < NBLK - 1 else ps_last
                    ps1 = dps.tile([128, 264], F32, tag="ps1")
                    for k in range(4):
                        nc.tensor.matmul(
                            out=ps1[:pb, :], lhsT=xT_t[k][:, P * nb:P * nb + pb],
                            rhs=rhsW1[k][:], start=(k == 0), stop=(k == 3))
                    combo = combop.tile([128, RW1], BF16, tag="combo1")
                    nc.vector.tensor_copy(combo[:pb, 0:256], ps1[:pb, 0:256])
                    nc.vector.tensor_copy(
                        combo[:pb, 256:264].bitcast(F32), ps1[:pb, 256:260])
                    # er -> persistent SBUF (bf16); pad rows of last block garbage-ok
                    nc.vector.tensor_copy(er1_sb[:pb, 4 * nb:4 * nb + 4], ps1[:pb, 260:264])
                    nc.sync.dma_start(
                        out=T1_local[P * nb:P * nb + pb, :], in_=combo[:pb, :])

            # ---------------- phase 2: allgather T1 ----------------
            if STAGE >= 2:
                nc.gpsimd.collective_compute(
                    "AllGather", mybir.AluOpType.bypass,
                    replica_groups=[list(range(NCORES))],
                    ins=[T1_local[:, :]], outs=[T1_full[:, :]])

            # ---------------- phase 3: L1 edge aggregation + inline dense L2 ----------
            with (
                tc.tile_pool(name="esb", bufs=EBUFS) as esb,
                tc.tile_pool(name="esb3", bufs=3) as esb3,
                tc.tile_pool(name="eps", bufs=2, space="PSUM") as eps,
                tc.tile_pool(name="tps", bufs=2, space="PSUM") as tps,
                tc.tile_pool(name="gps", bufs=2, space="PSUM") as gps,
                tc.tile_pool(name="d2sb", bufs=1) as d2sb,
                tc.tile_pool(name="d2ps", bufs=2, space="PSUM") as d2ps,
                tc.tile_pool(name="combop2", bufs=3) as combop2,
            ):
                # L2 dense weights (once)
                w2t_t = d2sb.tile([128, HD1], BF16, tag="w2t")
                alar2_t = d2sb.tile([128, 2], BF16, tag="alar2")
                nc.gpsimd.dma_start(out=w2t_t[:64, :], in_=w2t[:, :])
                nc.gpsimd.dma_start(out=alar2_t[:64, :], in_=alar2[:, :])
                rhsW2 = []
                for k in range(2):
                    rhsW2_k = d2sb.tile([128, 66], BF16, tag=f"rhsW2{k}", name=f"rhsW2{k}")
                    rhsW2.append(rhsW2_k)
                for k in range(2):
                    nc.gpsimd.dma_start(out=rhsW2[k][:, 0:64], in_=w2[128 * k:128 * (k + 1), :])
                    psw2 = d2ps.tile([128, 66], F32, tag="ps2")
                    nc.tensor.matmul(
                        out=psw2[:, 0:2], lhsT=w2t_t[:64, 128 * k:128 * (k + 1)],
                        rhs=alar2_t[:64, :], start=True, stop=True)
                    nc.vector.tensor_copy(rhsW2[k][:, 64:66], psw2[:, 0:2])

                GRP = int(os.environ.get("K_GRP", "2"))
                blks = list(range(NBLK if STAGE >= 3 else 0))
                # precompute per-group metadata
                gmetas = []
                clo = chi = ct = 0
                for i in range(0, len(blks), GRP):
                    grp = blks[i:i + GRP]
                    metas, toff = [], 0
                    for b in grp:
                        a, bb = int(nA[b]), int(nB[b])
                        metas.append((b, a, bb, a + bb, toff, clo, chi, ct))
                        toff += a + bb
                        clo += a * 8
                        chi += bb * 8
                        ct += a + bb
                    gmetas.append((metas, toff, metas[0][7]))

                def gathers1(gm):
                    metas, E_t, ct0 = gm
                    buf = esb3.tile([128, E_t * RW1], BF16, tag="buf")
                    for (b, a, bb, t_b, toff, clo_b, chi_b, _) in metas:
                        if a:
                            for c0 in range(0, a, GCH):
                                cn = min(GCH, a - c0)
                                nc.gpsimd.dma_gather(
                                    out_ap=buf[:, RW1 * (toff + c0):RW1 * (toff + c0 + cn)].rearrange("p (t e) -> p t e", e=RW1),
                                    in_ap=T1_full[0:LO, :],
                                    idxs_ap=ilo_t[:, clo_b + c0 * 8:clo_b + (c0 + cn) * 8],
                                    num_idxs=cn * P, num_idxs_reg=cn * P, elem_size=RW1,
                                    queue_num=nextq(), single_packet=SP)
                        if bb:
                            for c0 in range(0, bb, GCH):
                                cn = min(GCH, bb - c0)
                                nc.gpsimd.dma_gather(
                                    out_ap=buf[:, RW1 * (toff + a + c0):RW1 * (toff + a + c0 + cn)].rearrange("p (t e) -> p t e", e=RW1),
                                    in_ap=T1_full[LO:N, :],
                                    idxs_ap=ihi_t[:, chi_b + c0 * 8:chi_b + (c0 + cn) * 8],
                                    num_idxs=cn * P, num_idxs_reg=cn * P, elem_size=RW1,
                                    queue_num=nextq(), single_packet=SP)
                    return buf

                def head1(gm, buf):
                    metas, E_t, ct0 = gm
                    # S build: one batched is_equal over the whole group
                    S = esb.tile([128, E_t * 128], BF16, tag="S")
                    dsl = doff_t[:, ct0:ct0 + E_t]
                    d_b = bass.AP(dsl.tensor, dsl.offset, [dsl.ap[0], [1, E_t], [0, 128]])
                    i_b = bass.AP(iota_t[:].tensor, iota_t[:].offset,
                                  [iota_t[:].ap[0], [0, E_t], [1, 128]])
                    nc.vector.tensor_tensor(
                        out=S[:].rearrange("p (t c) -> p t c", c=128),
                        in0=i_b, in1=d_b, op=mybir.AluOpType.is_equal)

                    # S^T via PE transpose (8 tiles -> one PSUM bank -> copy)
                    ST = esb.tile([128, E_t * 128], BF16, tag="ST")
                    for g0 in range(0, E_t, 8):
                        gn = min(8, E_t - g0)
                        ptt = tps.tile([128, 1024], BF16, tag="ptt")
                        for k in range(gn):
                            nc.tensor.transpose(
                                out=ptt[:, 128 * k:128 * (k + 1)],
                                in_=S[:, 128 * (g0 + k):128 * (g0 + k + 1)], identity=ident[:])
                        if STC:
                            nc.scalar.activation(out=ST[:, 128 * g0:128 * (g0 + gn)],
                                                 in_=ptt[:, 0:128 * gn],
                                                 func=mybir.ActivationFunctionType.Copy)
                        else:
                            nc.vector.tensor_copy(ST[:, 128 * g0:128 * (g0 + gn)], ptt[:, 0:128 * gn])

                    # er per edge via one-hot expansion matmuls
                    G = gps.tile([128, E_t * 4], F32, tag="G")
                    for (b, a, bb, t_b, toff, _, _, _) in metas:
                        for t in range(t_b):
                            ti = toff + t
                            nc.tensor.matmul(
                                out=G[:, 4 * ti:4 * ti + 4], lhsT=ST[:, 128 * ti:128 * (ti + 1)],
                                rhs=er1_sb[:, 4 * b:4 * b + 4], start=True, stop=True)

                    # z = el_src + er_dst ; ex = max(exp(z), exp(.2 z))
                    z = esb.tile([128, E_t * 4], F32, tag="z")
                    bufv = buf[:].bitcast(F32).rearrange("p (t c) -> p t c", c=192)
                    nc.vector.tensor_tensor(
                        out=z[:].rearrange("p (t h) -> p t h", h=4),
                        in0=bufv[:, :, 128:132],
                        in1=G[:].rearrange("p (t h) -> p t h", h=4),
                        op=mybir.AluOpType.add)
                    e1 = esb.tile([128, E_t * 4], F32, tag="e1")
                    e2 = esb.tile([128, E_t * 4], F32, tag="e2")
                    nc.scalar.activation(out=e1[:], in_=z[:], func=mybir.ActivationFunctionType.Exp)
                    nc.scalar.activation(out=e2[:], in_=z[:], func=mybir.ActivationFunctionType.Exp, scale=0.2)
                    exb = esb.tile([128, E_t * 4], BF16, tag="exb")
                    nc.vector.tensor_tensor(out=exb[:], in0=e1[:], in1=e2[:], op=mybir.AluOpType.max)

                    # scale feat in place by ex; put ex into dead el slots ->
                    # contiguous 260-wide rhs [feat*ex | ex] per tile
                    br = buf[:].rearrange("p (t c) -> p t c", c=RW1)
                    for h in range(4):
                        es = exb[:, h:]
                        e_b = bass.AP(es.tensor, es.offset, [es.ap[0], [4, E_t], [0, 64]])
                        nc.vector.tensor_tensor(
                            out=br[:, :, 64 * h:64 * (h + 1)],
                            in0=br[:, :, 64 * h:64 * (h + 1)],
                            in1=e_b, op=mybir.AluOpType.mult)
                    nc.vector.tensor_copy(
                        br[:, :, 256:260], exb[:].rearrange("p (t h) -> p t h", h=4))
                    return S

                def tail1(gm, buf, S):
                    metas, E_t, ct0 = gm
                    nb = len(metas)
                    ps_os = []
                    for (b, a, bb, t_b, toff, _, _, _) in metas:
                        ps_o = eps.tile([128, 260], F32, tag="ps_o")
                        for t in range(t_b):
                            ti = toff + t
                            nc.tensor.matmul(
                                out=ps_o[:], lhsT=S[:, 128 * ti:128 * (ti + 1)],
                                rhs=buf[:, RW1 * ti:RW1 * ti + 260],
                                start=(t == 0), stop=(t == t_b - 1))
                        ps_os.append(ps_o)

                    # normalize into one group tile, then batched elu
                    xn = esb.tile([128, nb * 256], F32, tag="xn")
                    for j, (ps_o, m) in enumerate(zip(ps_os, metas)):
                        splus = esb.tile([128, 4], F32, tag="splus")
                        nc.vector.tensor_scalar(
                            out=splus[:], in0=ps_o[:, 256:260], scalar1=1e-30,
                            scalar2=None, op0=mybir.AluOpType.add)
                        r = esb.tile([128, 4], F32, tag="r")
                        nc.vector.reciprocal(r[:], splus[:])
                        r_b = bass.AP(r[:].tensor, r[:].offset, [r[:].ap[0], [1, 4], [0, 64]])
                        nc.vector.tensor_tensor(
                            out=xn[:, 256 * j:256 * (j + 1)].rearrange("p (h d) -> p h d", h=4),
                            in0=ps_o[:, 0:256].rearrange("p (h d) -> p h d", h=4),
                            in1=r_b, op=mybir.AluOpType.mult)
                    if has_b1:
                        b1b = bass.AP(b1_t[:].tensor, b1_t[:].offset,
                                      [b1_t[:].ap[0], [0, nb], [1, 256]])
                        nc.vector.tensor_tensor(
                            out=xn[:].rearrange("p (j c) -> p j c", c=256),
                            in0=xn[:].rearrange("p (j c) -> p j c", c=256),
                            in1=b1b, op=mybir.AluOpType.add)
                    # elu(x) = exp(min(x,0)) + (max(x,0) - 1), batched over the group
                    t1 = esb.tile([128, nb * 256], F32, tag="t1")
                    nc.vector.tensor_scalar(
                        out=t1[:], in0=xn[:], scalar1=0.0, scalar2=None, op0=mybir.AluOpType.min)
                    u = esb.tile([128, nb * 256], F32, tag="u")
                    nc.scalar.activation(out=u[:], in_=t1[:], func=mybir.ActivationFunctionType.Exp)
                    v = esb.tile([128, nb * 256], F32, tag="v")
                    nc.vector.tensor_scalar(
                        out=v[:], in0=xn[:], scalar1=0.0, scalar2=-1.0,
                        op0=mybir.AluOpType.max, op1=mybir.AluOpType.add)
                    hb = esb.tile([128, nb * 256], BF16, tag="hb")
                    nc.vector.tensor_tensor(out=hb[:], in0=u[:], in1=v[:], op=mybir.AluOpType.add)
                    # transpose all blocks' hb -> hT (contiguous dest cols)
                    b0 = metas[0][0]
                    for c0 in range(0, nb * 256, 1024):
                        cw = min(1024, nb * 256 - c0)
                        pst = tps.tile([128, 1024], BF16, tag="ptt")
                        for k2 in range(cw // 128):
                            nc.tensor.transpose(out=pst[:, 128 * k2:128 * (k2 + 1)],
                                                in_=hb[:, c0 + 128 * k2:c0 + 128 * (k2 + 1)], identity=ident[:])
                        nc.vector.tensor_copy(hT[:, 256 * b0 + c0:256 * b0 + c0 + cw], pst[:, 0:cw])

                    if STAGE >= 4:
                        for (b, a, bb, t_b, toff, _, _, _) in metas:
                            pb = P if b < NBLK - 1 else ps_last
                            ps2 = d2ps.tile([128, 66], F32, tag="ps2")
                            for k in range(2):
                                nc.tensor.matmul(
                                    out=ps2[:pb, :], lhsT=hT[:, 256 * b + 128 * k:256 * b + 128 * k + pb],
                                    rhs=rhsW2[k][:], start=(k == 0), stop=(k == 1))
                            combo2 = combop2.tile([128, RW2], BF16, tag="combo2")
                            nc.vector.tensor_copy(combo2[:pb, 0:64], ps2[:pb, 0:64])
                            nc.vector.tensor_copy(combo2[:pb, 64:66].bitcast(F32), ps2[:pb, 64:65])
                            nc.vector.tensor_copy(er2_sb[:pb, b:b + 1], ps2[:pb, 65:66])
                            nc.sync.dma_start(out=T2_local[P * b:P * b + pb, :], in_=combo2[:pb, :])

                # software-pipelined emission: gathers one group ahead of the
                # compute head; each group's tail after the next group's head
                if gmetas:
                    bufs = {0: gathers1(gmetas[0])}
                    Ss = {}
                    for i in range(len(gmetas)):
                        if i + 1 < len(gmetas):
                            bufs[i + 1] = gathers1(gmetas[i + 1])
                        Ss[i] = head1(gmetas[i], bufs[i])
                        if i >= 1:
                            tail1(gmetas[i - 1], bufs[i - 1], Ss[i - 1])
                            del bufs[i - 1], Ss[i - 1]
                    tail1(gmetas[-1], bufs[len(gmetas) - 1], Ss[len(gmetas) - 1])

            # ---------------- phase 5: allgather T2 ----------------
            if STAGE >= 5:
                nc.gpsimd.collective_compute(
                    "AllGather", mybir.AluOpType.bypass,
                    replica_groups=[list(range(NCORES))],
                    ins=[T2_local[:, :]], outs=[T2_full[:, :]])

            # ---------------- phase 6: L2 edge aggregation ----------------
            with (
                tc.tile_pool(name="e2sb", bufs=EBUFS) as e2sb,
                tc.tile_pool(name="e2sb3", bufs=3) as e2sb3,
                tc.tile_pool(name="e2ps", bufs=2, space="PSUM") as e2ps,
                tc.tile_pool(name="t2ps", bufs=2, space="PSUM") as t2ps,
                tc.tile_pool(name="g2ps", bufs=2, space="PSUM") as g2ps,
            ):
                GRP = int(os.environ.get("K_GRP", "2"))
                blks = list(range(NBLK if STAGE >= 6 else 0))
                gmetas = []
                clo = chi = ct = 0
                for i in range(0, len(blks), GRP):
                    grp = blks[i:i + GRP]
                    metas, toff = [], 0
                    for b in grp:
                        a, bb = int(nA[b]), int(nB[b])
                        metas.append((b, a, bb, a + bb, toff, clo, chi, ct))
                        toff += a + bb
                        clo += a * 8
                        chi += bb * 8
                        ct += a + bb
                    gmetas.append((metas, toff, metas[0][7]))

                def gathers2(gm):
                    metas, E_t, ct0 = gm
                    buf = e2sb3.tile([128, E_t * RW2], BF16, tag="buf2")
                    for (b, a, bb, t_b, toff, clo_b, chi_b, _) in metas:
                        if a:
                            for c0 in range(0, a, GCH):
                                cn = min(GCH, a - c0)
                                nc.gpsimd.dma_gather(
                                    out_ap=buf[:, RW2 * (toff + c0):RW2 * (toff + c0 + cn)].rearrange("p (t e) -> p t e", e=RW2),
                                    in_ap=T2_full[0:LO, :],
                                    idxs_ap=ilo_t[:, clo_b + c0 * 8:clo_b + (c0 + cn) * 8],
                                    num_idxs=cn * P, num_idxs_reg=cn * P, elem_size=RW2,
                                    queue_num=nextq(), single_packet=SP)
                        if bb:
                            for c0 in range(0, bb, GCH):
                                cn = min(GCH, bb - c0)
                                nc.gpsimd.dma_gather(
                                    out_ap=buf[:, RW2 * (toff + a + c0):RW2 * (toff + a + c0 + cn)].rearrange("p (t e) -> p t e", e=RW2),
                                    in_ap=T2_full[LO:N, :],
                                    idxs_ap=ihi_t[:, chi_b + c0 * 8:chi_b + (c0 + cn) * 8],
                                    num_idxs=cn * P, num_idxs_reg=cn * P, elem_size=RW2,
                                    queue_num=nextq(), single_packet=SP)
                    return buf

                def head2(gm, buf):
                    metas, E_t, ct0 = gm
                    S = e2sb.tile([128, E_t * 128], BF16, tag="S2")
                    dsl = doff_t[:, ct0:ct0 + E_t]
                    d_b = bass.AP(dsl.tensor, dsl.offset, [dsl.ap[0], [1, E_t], [0, 128]])
                    i_b = bass.AP(iota_t[:].tensor, iota_t[:].offset,
                                  [iota_t[:].ap[0], [0, E_t], [1, 128]])
                    nc.vector.tensor_tensor(
                        out=S[:].rearrange("p (t c) -> p t c", c=128),
                        in0=i_b, in1=d_b, op=mybir.AluOpType.is_equal)

                    ST = e2sb.tile([128, E_t * 128], BF16, tag="ST2")
                    for g0 in range(0, E_t, 8):
                        gn = min(8, E_t - g0)
                        ptt = t2ps.tile([128, 1024], BF16, tag="ptt2")
                        for k in range(gn):
                            nc.tensor.transpose(
                                out=ptt[:, 128 * k:128 * (k + 1)],
                                in_=S[:, 128 * (g0 + k):128 * (g0 + k + 1)], identity=ident[:])
                        if STC:
                            nc.scalar.activation(out=ST[:, 128 * g0:128 * (g0 + gn)],
                                                 in_=ptt[:, 0:128 * gn],
                                                 func=mybir.ActivationFunctionType.Copy)
                        else:
                            nc.vector.tensor_copy(ST[:, 128 * g0:128 * (g0 + gn)], ptt[:, 0:128 * gn])

                    G = g2ps.tile([128, E_t], F32, tag="G2")
                    for (b, a, bb, t_b, toff, _, _, _) in metas:
                        for t in range(t_b):
                            ti = toff + t
                            nc.tensor.matmul(
                                out=G[:, ti:ti + 1], lhsT=ST[:, 128 * ti:128 * (ti + 1)],
                                rhs=er2_sb[:, b:b + 1], start=True, stop=True)

                    z = e2sb.tile([128, E_t], F32, tag="z2")
                    bufv = buf[:].bitcast(F32).rearrange("p (t c) -> p t c", c=64)
                    nc.vector.tensor_tensor(
                        out=z[:].rearrange("p (t h) -> p t h", h=1),
                        in0=bufv[:, :, 32:33],
                        in1=G[:].rearrange("p (t h) -> p t h", h=1),
                        op=mybir.AluOpType.add)
                    e1 = e2sb.tile([128, E_t], F32, tag="e12")
                    e2 = e2sb.tile([128, E_t], F32, tag="e22")
                    nc.scalar.activation(out=e1[:], in_=z[:], func=mybir.ActivationFunctionType.Exp)
                    nc.scalar.activation(out=e2[:], in_=z[:], func=mybir.ActivationFunctionType.Exp, scale=0.2)
                    exb = e2sb.tile([128, E_t], BF16, tag="exb2")
                    nc.vector.tensor_tensor(out=exb[:], in0=e1[:], in1=e2[:], op=mybir.AluOpType.max)

                    br = buf[:].rearrange("p (t c) -> p t c", c=RW2)
                    e_b = bass.AP(exb[:].tensor, exb[:].offset, [exb[:].ap[0], [1, E_t], [0, 64]])
                    nc.vector.tensor_tensor(
                        out=br[:, :, 0:64], in0=br[:, :, 0:64], in1=e_b,
                        op=mybir.AluOpType.mult)
                    nc.vector.tensor_copy(
                        br[:, :, 64:65], exb[:].rearrange("p (t h) -> p t h", h=1))
                    return S

                def tail2(gm, buf, S):
                    metas, E_t, ct0 = gm
                    for (b, a, bb, t_b, toff, _, _, _) in metas:
                        pb = P if b < NBLK - 1 else ps_last
                        ps_o = e2ps.tile([128, 65], F32, tag="ps_o2")
                        for t in range(t_b):
                            ti = toff + t
                            nc.tensor.matmul(
                                out=ps_o[:], lhsT=S[:, 128 * ti:128 * (ti + 1)],
                                rhs=buf[:, RW2 * ti:RW2 * ti + 65],
                                start=(t == 0), stop=(t == t_b - 1))

                        splus = e2sb.tile([128, 1], F32, tag="splus2")
                        nc.vector.tensor_scalar(
                            out=splus[:], in0=ps_o[:, 64:65], scalar1=1e-30,
                            scalar2=None, op0=mybir.AluOpType.add)
                        r = e2sb.tile([128, 1], F32, tag="r2")
                        nc.vector.reciprocal(r[:], splus[:])
                        outf = e2sb.tile([128, 64], F32, tag="outf")
                        nc.vector.tensor_scalar(
                            out=outf[:], in0=ps_o[:, 0:64], scalar1=r[:, 0:1],
                            scalar2=None, op0=mybir.AluOpType.mult)
                        if has_b2:
                            nc.vector.tensor_tensor(out=outf[:], in0=outf[:], in1=b2_t[:], op=mybir.AluOpType.add)
                        nc.sync.dma_start(out=out_t[P * b:P * b + pb, :], in_=outf[:pb, :])

                if gmetas:
                    bufs = {0: gathers2(gmetas[0])}
                    Ss = {}
                    for i in range(len(gmetas)):
                        if i + 1 < len(gmetas):
                            bufs[i + 1] = gathers2(gmetas[i + 1])
                        Ss[i] = head2(gmetas[i], bufs[i])
                        if i >= 1:
                            tail2(gmetas[i - 1], bufs[i - 1], Ss[i - 1])
                            del bufs[i - 1], Ss[i - 1]
                    tail2(gmetas[-1], bufs[len(gmetas) - 1], Ss[len(gmetas) - 1])

            if STAGE < 6:
                zer = cpool.tile([128, D2], F32, name="zer")
                nc.vector.memset(zer[:], 0)
                for b in range(NBLK):
                    pb = P if b < NBLK - 1 else ps_last
                    nc.sync.dma_start(out=out_t[P * b:P * b + pb, :], in_=zer[:pb, :])

    nc.compile()
    return nc


def kernel(x, src, dst, W1, al1, ar1, b1, W2, al2, ar2, b2):
    x = np.asarray(x, dtype=np.float32)
    W1 = np.asarray(W1, dtype=np.float32)
    al1 = np.asarray(al1, dtype=np.float32)
    ar1 = np.asarray(ar1, dtype=np.float32)
    b1 = np.asarray(b1, dtype=np.float32)
    W2 = np.asarray(W2, dtype=np.float32)
    al2 = np.asarray(al2, dtype=np.float32)
    ar2 = np.asarray(ar2, dtype=np.float32)
    b2 = np.asarray(b2, dtype=np.float32)

    nA, nB, NT, idx_lo, idx_hi, doff = _prep_edges(src, dst)
    CL, CH = idx_lo[0].shape[1], idx_hi[0].shape[1]
    has_b1 = bool(np.any(b1))
    has_b2 = bool(np.any(b2))

    nc = _build(nA, nB, NT, CL, CH, has_b1, has_b2)

    alar1_np = np.zeros((HD1, 8), np.float32)
    for h in range(H1):
        alar1_np[64 * h:64 * (h + 1), h] = al1[h]
        alar1_np[64 * h:64 * (h + 1), 4 + h] = ar1[h]
    alar2_np = np.zeros((D2, 2), np.float32)
    alar2_np[:, 0] = al2[0]
    alar2_np[:, 1] = ar2[0]
    w1t_np = np.ascontiguousarray(W1.T)
    w2t_np = np.ascontiguousarray(W2.T)
    xt_np = np.ascontiguousarray(x.T)

    in_maps = []
    for d in range(NCORES):
        m = {
            "xT": np.ascontiguousarray(xt_np[:, NSH * d:NSH * (d + 1)]),
            "w1": W1, "w1t": w1t_np, "alar1": alar1_np,
            "w2": W2, "w2t": w2t_np, "alar2": alar2_np,
            "ilo": np.ascontiguousarray(idx_lo[d]) if CL else np.zeros((128, 1), np.int16),
            "ihi": np.ascontiguousarray(idx_hi[d]) if CH else np.zeros((128, 1), np.int16),
            "idoff": np.ascontiguousarray(doff[d]),
        }
        if has_b1:
            m["b1r"] = np.tile(b1.reshape(1, HD1), (128, 1)).astype(np.float32)
        if has_b2:
            m["b2r"] = np.tile(b2.reshape(1, D2), (128, 1)).astype(np.float32)
        in_maps.append(m)

    res = run_bass_kernel_spmd(nc, in_maps, core_ids=list(range(NCORES)))
    out = np.concatenate([res.results[d]["out"] for d in range(NCORES)], axis=0)
    return out
